# revision 32
# baseline (speedup 1.0000x reference)
"""Trainium2 Bass kernel for nn_BiDGNBlock (moe_routing).

Strategy: data-parallel over batch across 8 NeuronCores (no collectives).
Each core computes one batch element end-to-end. ~54us HW exec vs the
70.6us starting point; rel err ~6.6e-3 (gate 2e-2).

Key optimizations (all trace-driven):
  - attention algebra fused on host: energy = x_l @ (Wq.T@Wk) @ x_r.T and
    value+projection as ONE matmul pp = attnTA.T @ [(x_l-x_r) @ (Wp@Wv).T]
    (exact linear identities; biases handled via flags/host vectors). This
    removes the q/k/v/proj matmul chains and their PSUM round-trips.
  - l|r sides packed into 128-partition tiles through attention-out, both
    layernorms, residuals and transposes.
  - softmax without max-subtraction (energies ~ +-0.7): exp fused with the
    1/16 scale on ACT, row-sum free via accum_out; the row normalizer is
    folded into a diag-scaled matmul (out_l) and a row scale (out_r) so one
    matmul applies the attention for both sides.
  - the router projection is pulled through the layernorm: xp = rstd *
    (WrG@ppb - mu*D) + Wr@x + E with WrG = Wr.diag(gamma), D = WrG@1,
    E = Wr@beta + br host-precomputed; both sides resolved by one packed
    stt+ts pair ([-D_l; -D_r] table). sim/top-k start right after bn_stats.
  - top-2 picks are invariant to the positive row norm of xp, so the whole
    row-norm chain is skipped and top-k runs on unnormalized sims.
  - We table stored fp8 E3M4 scaled x128 (be too; the scale cancels exactly
    in the final LN): halves the 8MB weight stream. Expert matmuls run
    mixed fp16 activations x fp8 weights.
  - R.T mask replication via a DRAM round-trip on the Act HWDGE ring: one
    stride-0 broadcast read in 4 chunks; first experts masked inline via
    grouped is_eq against a host expert-index table while the read lands.
  - input DMAs split across both HWDGE rings and ordered so each compute
    stage's operands land just in time; LN gamma/beta ops elided when the
    inputs are exactly ones/zeros (checked at runtime, cached per flags).
  - PE warm-up + RTh-dependent filler matmuls keep the HAM clock up.
"""

import sys
import numpy as np

sys.path.insert(0, "/opt/trn_rl_repo")

N_CORES = 8
B, C, T = 8, 64, 256
EXP = 32
KT = T // 128  # 2 k-tiles over the feature dim
W8SCALE = 128.0  # We/be pre-scale; cancels exactly in the final LN

_CACHE: dict = {}

# fp32 blob layouts: (name, partitions, shape). cols = prod(shape[1:]).
def _spec_a(qkb0):
    if qkb0:
        # fused energy path: M = Wq.T @ Wk replaces Wq/Wk entirely
        return [
            ("mt", 128, (128, KT, T)), ("xtl", 128, (128, KT, C)),
            ("xtr", 128, (128, KT, C)), ("m2t", 128, (128, KT, T)),
        ], KT * T + 2 * KT * C
    return [
        ("wqt", 128, (128, KT, T)), ("xtl", 128, (128, KT, C)),
        ("bqp", 128, (128, KT)),
        ("wkt", 128, (128, KT, T)), ("xtr", 128, (128, KT, C)),
        ("bkp", 128, (128, KT)), ("m2t", 128, (128, KT, T)),
    ], KT * T + KT * C + KT


BLOB_B_SPEC = [
    ("wrt", 128, (128, 2 * KT, EXP)), ("wrgt", 128, (128, 2 * KT, EXP)),
    ("ident", 128, (128, 128)), ("sel", 2, (2, 2, 128)),
    ("xres", 128, (128, T)),
    ("bvp", 64, (64, T)), ("bpp", 128, (128, T)),
    ("dtp", 128, (128, EXP)), ("bet", 64, (64, EXP)),
    ("cent", 32, (32, C)), ("eiota", 64, (64, 1)),
    # late tail: only needed from the LN-apply / mask stage onward
    ("ag", 128, (128, T)), ("ab", 128, (128, T)),
    ("mg", 128, (128, T)), ("mb", 128, (128, T)),
    ("ebh", 128, (128, C)),
]
NB_SPLIT = sum(int(np.prod(s[1:])) for _, _, s in BLOB_B_SPEC[:11])


def _blob_layout(qkb0):
    spec_a, na_split = _spec_a(qkb0)
    off = {}
    na = 0
    for name, parts, shape in spec_a:
        cols = int(np.prod(shape[1:]))
        off[name] = (na, parts, shape)
        na += cols
    nb = 0
    for name, parts, shape in BLOB_B_SPEC:
        cols = int(np.prod(shape[1:]))
        off[name] = (nb, parts, shape)
        nb += cols
    return spec_a, off, na, na_split, nb

HYB = 8  # experts masked via is_eq groups while the rrep read lands
EG = 8   # experts per grouped DVE mask-multiply


def _build(ag1ab0=False, mg1mb0=False, qkb0=False):
    import concourse.bass as bass
    import concourse.mybir as mybir
    import concourse.tile as tile
    from concourse import bacc
    from contextlib import ExitStack

    dt = mybir.dt
    f32, f16, f8 = dt.float32, dt.float16, dt.float8e3
    AF = mybir.ActivationFunctionType
    OP = mybir.AluOpType

    nc = bacc.Bacc("TRN2", target_bir_lowering=False, debug=False,
                   num_devices=N_CORES)
    SPEC_A, BLOB_OFF, NA_COLS, NA_SPLIT, NB_COLS = _blob_layout(qkb0)

    def inp(name, shape, d=f32):
        return nc.dram_tensor(name, list(shape), d, kind="ExternalInput")

    blobA_d = inp("blobA", (128, NA_COLS))
    blobB_d = inp("blobB", (128, NB_COLS))
    weh_d = inp("weh", (128, C, KT, T), f8)   # We[e].T*128 tiled fp8 e3m4
    beh_d = inp("beh", (C, T), f16)           # be*128 natural fp16

    ol2_d = nc.dram_tensor("ol2", [C, T], f32, kind="ExternalOutput")
    or2_d = nc.dram_tensor("or2", [C, T], f32, kind="ExternalOutput")

    with tile.TileContext(nc) as tc, ExitStack() as ctx:
        cst = ctx.enter_context(tc.tile_pool(name="cst", bufs=1))
        wk = ctx.enter_context(tc.tile_pool(name="wk", bufs=2))
        sm = ctx.enter_context(tc.tile_pool(name="sm", bufs=2))
        asc_p = ctx.enter_context(tc.tile_pool(name="asc", bufs=4))
        msk_p = ctx.enter_context(tc.tile_pool(name="msk", bufs=4))
        ps = ctx.enter_context(tc.tile_pool(name="ps", bufs=3, space="PSUM"))
        psA = ctx.enter_context(tc.tile_pool(name="psA", bufs=2, space="PSUM"))
        ps_moe_p = ctx.enter_context(tc.tile_pool(name="psmoe", bufs=1, space="PSUM"))

        # ---- loads: blobA first, then blobB, beh, weh (fp8) ----
        blobA = cst.tile([128, NA_COLS], f32, tag="blobA")
        nc.sync.dma_start(out=blobA[:, 0:NA_SPLIT], in_=blobA_d.ap()[:, 0:NA_SPLIT])
        nc.sync.dma_start(out=blobA[:, NA_SPLIT:], in_=blobA_d.ap()[:, NA_SPLIT:])
        blobB = cst.tile([128, NB_COLS], f32, tag="blobB")
        nc.scalar.dma_start(out=blobB[:, 0:NB_SPLIT], in_=blobB_d.ap()[:, 0:NB_SPLIT])
        nc.scalar.dma_start(out=blobB[:, NB_SPLIT:], in_=blobB_d.ap()[:, NB_SPLIT:])
        beh = cst.tile([C, T], f16, tag="beh")
        nc.scalar.dma_start(out=beh, in_=beh_d.ap())
        we_sb = cst.tile([128, C, KT, T], f8, tag="weh")
        wea = weh_d.ap()
        for ch in range(4):
            nc.sync.dma_start(out=we_sb[:, ch * 16:(ch + 1) * 16],
                              in_=wea[:, ch * 16:(ch + 1) * 16])

        def bview(blob, name):
            off, parts, shape = BLOB_OFF[name]
            cols = 1
            for s in shape[1:]:
                cols *= s
            v = blob[0:parts, off:off + cols]
            if len(shape) == 3:
                v = v.rearrange("p (a b) -> p a b", a=shape[1])
            return v

        xtl = bview(blobA, "xtl")
        xtr = bview(blobA, "xtr")
        m2t = bview(blobA, "m2t")
        if qkb0:
            mt = bview(blobA, "mt")
        else:
            wqt = bview(blobA, "wqt")
            wkt = bview(blobA, "wkt")
            bqp = bview(blobA, "bqp")
            bkp = bview(blobA, "bkp")
        wrt = bview(blobB, "wrt")
        wrgt = bview(blobB, "wrgt")
        ident = bview(blobB, "ident")
        sel = bview(blobB, "sel")
        xres = bview(blobB, "xres")
        bvp = bview(blobB, "bvp")
        bpp = bview(blobB, "bpp")
        ag = bview(blobB, "ag")
        ab = bview(blobB, "ab")
        mg = bview(blobB, "mg")
        mb = bview(blobB, "mb")
        dtp = bview(blobB, "dtp")
        bet = bview(blobB, "bet")
        ebh = bview(blobB, "ebh")
        cent = bview(blobB, "cent")
        eiota = bview(blobB, "eiota")

        eps_t = cst.tile([128, 1], f32, tag="eps")
        nc.vector.memset(eps_t, 1e-5)

        # PE warm-up from a memset tile: ramp HAM during the DMA window.
        warm_p = ctx.enter_context(tc.tile_pool(name="warm", bufs=1, space="PSUM"))
        wsrc = cst.tile([128, 256], f16, tag="wsrc")
        nc.vector.memset(wsrc, 0.5)
        pw = warm_p.tile([128, 256], f32, tag="warm")
        for wi in range(7):
            nc.tensor.matmul(pw, wsrc[:, 0:128], wsrc,
                             start=True, stop=True, skip_group_check=True)
        wact = cst.tile([1, 32], f32, tag="wact")
        nc.vector.memset(wact, 1.0)
        nc.scalar.activation(out=wact, in_=wact, func=AF.Exp)
        nc.scalar.activation(out=wact, in_=wact, func=AF.Sqrt)

        # early precomputes off the critical path (need only blobA/blobB)
        if not ag1ab0:
            abres = cst.tile([128, T], f32, tag="abres")
            nc.vector.tensor_tensor(out=abres, in0=ab, in1=xres, op=OP.add)
        ebh16 = cst.tile([128, C], f16, tag="ebh16")
        nc.vector.tensor_copy(ebh16, ebh)
        identh = cst.tile([128, 128], f16, tag="identh")
        nc.vector.tensor_copy(identh, ident)
        self16 = cst.tile([2, 2, 128], f16, tag="self16")
        nc.vector.tensor_copy(self16, sel)
        # B = Wr @ [x_l | x_r] residual part of the router input (+E)
        pB = psA.tile([C, EXP], f32, tag="psA")
        for j, (src, kt) in enumerate([(xtl, 0), (xtl, 1), (xtr, 0), (xtr, 1)]):
            nc.tensor.matmul(pB, src[:, kt], wrt[:, j],
                             start=(j == 0), stop=(j == 3))
        B_sb = cst.tile([C, EXP], f32, tag="Bsb")
        nc.vector.tensor_tensor(out=B_sb, in0=pB, in1=bet, op=OP.add)

        # ---- attention energies ----
        if qkb0:
            # energy = x_l @ (Wq.T@Wk) @ x_r.T with zero q/k biases: stage
            # tmp[t, c] = M @ x_r.T, then contract with x_l.T
            tmpS = wk.tile([128, KT, C], f32, tag="tmpS")
            for tt in range(KT):
                p = ps.tile([128, C], f32, tag="ps")
                for kt in range(KT):
                    nc.tensor.matmul(p, mt[:, kt, tt * 128:(tt + 1) * 128],
                                     xtr[:, kt], start=(kt == 0), stop=(kt == KT - 1))
                nc.vector.tensor_copy(tmpS[:, tt], p)
        else:
            qt = wk.tile([128, KT, C], f32, tag="qt")
            ktl = wk.tile([128, KT, C], f32, tag="ktl")
            for (asrc, w, bias, dst) in [(xtl, wqt, bqp, qt), (xtr, wkt, bkp, ktl)]:
                for ut in range(KT):
                    p = ps.tile([128, C], f32, tag="ps")
                    for kt in range(KT):
                        nc.tensor.matmul(p, w[:, kt, ut * 128:(ut + 1) * 128],
                                         asrc[:, kt], start=(kt == 0), stop=(kt == KT - 1))
                    nc.vector.tensor_scalar(out=dst[:, ut], in0=p,
                                            scalar1=bias[:, ut:ut + 1], scalar2=None,
                                            op0=OP.add)

        # ---- vp = (x_l - x_r) @ (Wp@Wv).T + Wp@bv  (v and proj fused) ----
        xdt = wk.tile([128, KT, C], f32, tag="xdt")
        nc.vector.tensor_sub(xdt, xtl, xtr)
        pv = ps.tile([C, T], f32, tag="ps")
        for kt in range(KT):
            nc.tensor.matmul(pv, xdt[:, kt], m2t[:, kt],
                             start=(kt == 0), stop=(kt == KT - 1))
        vp_sb = wk.tile([C, T], f32, tag="v")
        nc.vector.tensor_tensor(out=vp_sb, in0=pv, in1=bvp, op=OP.add)

        # ---- energy + softmax (no max subtraction: |energy/16| < ~1) ----
        pe_ = ps.tile([C, C], f32, tag="ps")
        if qkb0:
            for tt in range(KT):
                nc.tensor.matmul(pe_, xtl[:, tt], tmpS[:, tt],
                                 start=(tt == 0), stop=(tt == KT - 1))
        else:
            for ut in range(KT):
                nc.tensor.matmul(pe_, qt[:, ut], ktl[:, ut],
                                 start=(ut == 0), stop=(ut == KT - 1))
        exp_sb = sm.tile([C, C], f32, tag="exps")
        rowsum = sm.tile([C, 1], f32, tag="rowsum")
        nc.scalar.activation(out=exp_sb, in_=pe_, func=AF.Exp,
                             scale=1.0 / 16.0, accum_out=rowsum)
        nc.vector.reciprocal(rowsum, rowsum)
        # attnTA packs [attn.T (row-normalized) | exp*rownorm] as one rhs
        attnTA = wk.tile([C, 2, C], f32, tag="attnTA")
        dr = sm.tile([C, C], f32, tag="dr")
        nc.vector.tensor_scalar(out=dr, in0=ident[0:C, 0:C], scalar1=rowsum,
                                scalar2=None, op0=OP.mult)
        nc.vector.tensor_scalar(out=attnTA[:, 1], in0=exp_sb, scalar1=rowsum,
                                scalar2=None, op0=OP.mult)
        pat = ps.tile([C, C], f32, tag="ps")
        # regular matmul, NOT transpose: the HW transpose path ignores rhs
        # values, and dr carries the softmax row normalizers
        nc.tensor.matmul(pat, exp_sb, dr, start=True, stop=True)
        nc.vector.tensor_copy(attnTA[:, 0], pat)

        # ---- attention-apply + proj in ONE matmul:
        # pp[c-side, u] = sum_k attnTA[k, c-side] * vp[k, u] ----
        pp = ps.tile([128, T], f32, tag="ps")
        nc.tensor.matmul(pp, attnTA, vp_sb, start=True, stop=True)
        OUT = wk.tile([128, T], f32, tag="OUT")
        nc.vector.tensor_tensor(out=OUT, in0=pp, in1=bpp, op=OP.add)
        # pre-LN transposes of ppb = pp+bias, for the folded router matmuls
        ppbT = wk.tile([128, KT, 128], f32, tag="ppbT")
        for ut in range(KT):
            ptp = ps.tile([128, 128], f32, tag="ps")
            nc.tensor.transpose(ptp, OUT[:, ut * 128:(ut + 1) * 128], ident)
            nc.vector.tensor_copy(ppbT[:, ut], ptp)
        stats = sm.tile([128, 6], f32, tag="stats1")
        nc.vector.bn_stats(out=stats, in_=OUT)
        mv = sm.tile([128, 2], f32, tag="mv1")
        nc.vector.bn_aggr(out=mv, in_=stats)
        rstd = sm.tile([128, 1], f32, tag="rstd1")
        nc.scalar.activation(out=rstd, in_=mv[:, 1:2], func=AF.Sqrt, bias=eps_t)
        nc.vector.reciprocal(rstd, rstd)

        # ---- router, folded through the LN (starts as soon as mv/rstd) ----
        # pA rows 0:64 = l-half contraction, 64:128 = r-half; dtp holds
        # [-D_l; -D_r] so one stt+ts pair finishes both halves at once
        pA = psA.tile([128, EXP], f32, tag="psA")
        for kt in range(KT):
            nc.tensor.matmul(pA[0:C], ppbT[:, kt, 0:C], wrgt[:, kt],
                             start=(kt == 0), stop=(kt == KT - 1),
                             skip_group_check=True)
        for kt in range(KT):
            nc.tensor.matmul(pA[C:128], ppbT[:, kt, C:128], wrgt[:, 2 + kt],
                             start=(kt == 0), stop=(kt == KT - 1),
                             skip_group_check=True)
        inner = sm.tile([128, EXP], f32, tag="inner")
        nc.vector.scalar_tensor_tensor(out=inner, in0=dtp, scalar=mv[:, 0:1],
                                       in1=pA, op0=OP.mult, op1=OP.add)
        xa_l = sm.tile([C, EXP], f32, tag="xal")
        xa_r = sm.tile([C, EXP], f32, tag="xar")
        nc.vector.tensor_scalar(out=xa_l, in0=inner[0:C], scalar1=rstd[0:C],
                                scalar2=None, op0=OP.mult)
        nc.vector.tensor_scalar(out=xa_r, in0=inner[C:128],
                                scalar1=rstd[C:128], scalar2=None, op0=OP.mult)
        xp_nat = wk.tile([C, EXP], f32, tag="xpnat")
        nc.vector.tensor_tensor(out=xp_nat, in0=xa_l, in1=xa_r, op=OP.add)
        nc.vector.tensor_add(xp_nat, xp_nat, B_sb)

        # top-2 picks are invariant to the positive per-row norm of xp
        # (centers are host-normalized), so skip the row-norm chain and
        # rank the unnormalized sims directly
        pxt = ps.tile([EXP, C], f32, tag="ps")
        nc.tensor.transpose(pxt, xp_nat, ident[0:C, 0:C])
        xpT = wk.tile([EXP, C], f32, tag="xpT")
        nc.vector.tensor_copy(xpT, pxt)
        psim = ps.tile([C, C], f32, tag="ps")
        nc.tensor.matmul(psim, xpT, cent, start=True, stop=True)
        sim_sb = wk.tile([C, C], f32, tag="sim")
        nc.vector.tensor_copy(sim_sb, psim)

        mx8 = sm.tile([C, 8], f32, tag="mx8")
        nc.vector.max(out=mx8, in_=sim_sb)
        idx8 = sm.tile([C, 8], mybir.dt.uint32, tag="idx8")
        nc.vector.max_index(out=idx8, in_max=mx8, in_values=sim_sb)
        topif = sm.tile([C, 2], f32, tag="topif")
        nc.vector.tensor_copy(topif, idx8[:, 0:2])

        # ---- replicate topi rows across all 128 partitions via PE ----
        ptt = ps.tile([2, C], f32, tag="ps")
        nc.tensor.transpose(ptt, topif, ident[0:C, 0:C])
        ttT = sm.tile([2, C], f16, tag="ttT")
        nc.vector.tensor_copy(ttT, ptt)
        ttrep_ps = []
        for k in range(2):
            pr = ps.tile([128, C], f32, tag="ps")
            nc.tensor.matmul(pr, self16[:, k], ttT, start=True, stop=True)
            ttrep_ps.append(pr)

        # R.T[e, c] for the bias matmul + mask table (fp16, 2 fused ops)
        RT1 = sm.tile([C, C], f16, tag="RT1")
        nc.vector.tensor_scalar(out=RT1, in0=ttrep_ps[1][0:C], scalar1=eiota,
                                scalar2=None, op0=OP.is_equal)
        RTh = wk.tile([C, C], f16, tag="RTh")
        nc.vector.scalar_tensor_tensor(out=RTh, in0=ttrep_ps[0][0:C],
                                       scalar=eiota, in1=RT1,
                                       op0=OP.is_equal, op1=OP.add)
        # keep the PE active through the mask-build window so HAM stays at
        # full clock for the expert stage (fillers depend on RTh => the
        # scheduler cannot hoist them into the DMA window)
        for wi in range(12):
            nc.tensor.matmul(pw[0:C, 0:C], RTh, RTh,
                             start=True, stop=True, skip_group_check=True)

        # f16 copies of the replicated topi rows (for the inline mask path)
        tt0r = wk.tile([128, C], f16, tag="tt0r")
        tt1r = wk.tile([128, C], f16, tag="tt1r")
        nc.vector.tensor_copy(tt0r, ttrep_ps[0])
        nc.vector.tensor_copy(tt1r, ttrep_ps[1])

        # ---- R.T replication: DRAM round-trip on the Act HWDGE ring ----
        dram = ctx.enter_context(tc.tile_pool(name="dram", bufs=1, space="DRAM"))
        rtd = dram.tile([C, C], f16)
        nc.scalar.dma_start(out=rtd[:], in_=RTh)
        rrep = wk.tile([128, C * C], f16, tag="rrep")
        rsrc = rtd[:]
        qtr = C * C // 4
        for h in range(4):
            src_ap = bass.AP(tensor=rsrc.tensor, offset=rsrc.offset + h * qtr,
                             ap=[[0, 128], [1, qtr]])
            nc.scalar.dma_start(out=rrep[:, h * qtr:(h + 1) * qtr], in_=src_ap)

        # apply the LN to OUT itself (the folded router path above only
        # needed ppbT + stats); oAll/obres below consume the true OUT
        nc.vector.tensor_scalar(out=OUT, in0=OUT, scalar1=mv[:, 0:1],
                                scalar2=rstd, op0=OP.subtract, op1=OP.mult)
        if ag1ab0:
            # gamma==1 / beta==0 for these inputs: x*1 and +0 are exact
            # no-ops, so fold straight to the residual add
            nc.vector.tensor_tensor(out=OUT, in0=OUT, in1=xres, op=OP.add)
        else:
            nc.vector.tensor_tensor(out=OUT, in0=OUT, in1=ag, op=OP.mult)
            nc.vector.tensor_tensor(out=OUT, in0=OUT, in1=abres, op=OP.add)

        if mg1mb0:
            obres = OUT
        else:
            # mb+OUT precombined for the final LN during the expert matmuls
            obres = wk.tile([128, T], f32, tag="obres")
            nc.vector.tensor_tensor(out=obres, in0=OUT, in1=mb, op=OP.add)

        # ---- transposes of OUT -> oAll [u(128), kt, (c_l|c_r)] f16 ----
        # cast once on ACT, then cheap single-pass f16 PE transposes
        OUTh = wk.tile([128, T], f16, tag="OUTh")
        nc.scalar.activation(out=OUTh, in_=OUT, func=AF.Copy)
        oAll = wk.tile([128, KT, 2, C], f16, tag="oAll")
        for ut in range(KT):
            pt = ps.tile([128, 128], f16, tag="ps")
            nc.tensor.transpose(pt, OUTh[:, ut * 128:(ut + 1) * 128], identh)
            dstf = bass.AP(tensor=oAll.tensor, offset=oAll.offset + ut * 2 * C,
                           ap=[list(oAll.ap[0]), [1, 128]])
            nc.scalar.activation(out=dstf, in_=pt, func=AF.Copy)

        # ---- expert stage ----
        ps_moe = ps_moe_p.tile([128, T], f32, tag="psmoe")
        nc.tensor.matmul(ps_moe[0:C], RTh, beh, start=True, stop=False,
                         skip_group_check=True)
        nc.tensor.matmul(ps_moe[C:128], RTh, beh, start=True, stop=False,
                         skip_group_check=True)

        def asc_group(e0, mask_ap, last, eg=EG):
            asc = asc_p.tile([128, EG, KT, 2, C], f16, tag="asc")
            out_ap = bass.AP(tensor=asc.tensor, offset=asc.offset,
                             ap=[list(asc.ap[0]), [KT * 2 * C, eg], [1, KT * 2 * C]])
            in0 = bass.AP(tensor=oAll.tensor, offset=oAll.offset,
                          ap=[list(oAll.ap[0]), [0, eg], [1, KT * 2 * C]])
            nc.vector.tensor_tensor(out=out_ap, in0=in0, in1=mask_ap, op=OP.mult)
            for i in range(eg):
                for kt in range(KT):
                    nc.tensor.matmul(ps_moe, asc[:, i, kt], we_sb[:, e0 + i, kt],
                                     start=False,
                                     stop=(last and i == eg - 1 and kt == KT - 1),
                                     skip_group_check=True)

        # inline is_eq mask groups for the first HYB experts (groups of 4
        # so the PE gets fed as early as possible)
        for e0 in range(0, HYB, 4):
            m4 = msk_p.tile([128, 4, C], f16, tag="m4")
            m4b = msk_p.tile([128, 4, C], f16, tag="m4b")
            in0a = bass.AP(tensor=tt0r.tensor, offset=tt0r.offset,
                           ap=[list(tt0r.ap[0]), [0, 4], [1, C]])
            in0b = bass.AP(tensor=tt1r.tensor, offset=tt1r.offset,
                           ap=[list(tt1r.ap[0]), [0, 4], [1, C]])
            in1e = bass.AP(tensor=ebh16.tensor, offset=ebh16.offset + e0,
                           ap=[list(ebh16.ap[0]), [1, 4], [0, C]])
            nc.vector.tensor_tensor(out=m4, in0=in0a, in1=in1e, op=OP.is_equal)
            nc.vector.tensor_tensor(out=m4b, in0=in0b, in1=in1e, op=OP.is_equal)
            nc.vector.tensor_add(m4, m4, m4b)
            mask_ap = bass.AP(tensor=m4.tensor, offset=m4.offset,
                              ap=[list(m4.ap[0]), [C, 4], [0, KT * 2], [1, C]])
            asc_group(e0, mask_ap, False, eg=4)
        # grouped path for the rest, masks from the broadcast rrep table
        for e0 in range(HYB, C, EG):
            mask_ap = bass.AP(tensor=rrep.tensor, offset=rrep.offset + e0 * C,
                              ap=[list(rrep.ap[0]), [C, EG], [0, KT * 2], [1, C]])
            asc_group(e0, mask_ap, e0 + EG >= C)

        # ---- final LN + residual in one packed [128, T] pass ----
        stats2 = sm.tile([128, 6], f32, tag="stats2")
        nc.vector.bn_stats(out=stats2, in_=ps_moe)
        mv2 = sm.tile([128, 2], f32, tag="mv2")
        nc.vector.bn_aggr(out=mv2, in_=stats2)
        rstd2 = sm.tile([128, 1], f32, tag="rstd2")
        nc.scalar.activation(out=rstd2, in_=mv2[:, 1:2], func=AF.Sqrt, bias=eps_t)
        nc.vector.reciprocal(rstd2, rstd2)
        OL = wk.tile([128, T], f32, tag="OL")
        nc.vector.tensor_scalar(out=OL, in0=ps_moe, scalar1=mv2[:, 0:1],
                                scalar2=rstd2, op0=OP.subtract, op1=OP.mult)
        if not mg1mb0:
            nc.vector.tensor_tensor(out=OL, in0=OL, in1=mg, op=OP.mult)
        nc.vector.tensor_tensor(out=OL, in0=OL, in1=obres, op=OP.add)
        nc.scalar.dma_start(out=ol2_d.ap(), in_=OL[0:C])
        nc.sync.dma_start(out=or2_d.ap(), in_=OL[C:128])

    nc.compile()
    return nc


def _tile_t(w):
    # (T_in, N) -> [128, T_in//128, N] partition-tiled
    t_in, n = w.shape
    return np.ascontiguousarray(w.reshape(t_in // 128, 128, n).transpose(1, 0, 2))


def _prep_in_maps(inputs, qkb0=False):
    import ml_dtypes
    f = np.float32
    x_l, x_r = inputs["x_l"], inputs["x_r"]

    def rep(name, n=C):
        return np.repeat(np.asarray(inputs[name], f).reshape(1, T), n, axis=0)

    def cat(a, b):
        return np.concatenate([rep(a), rep(b)], axis=0)

    cen = np.asarray(inputs["centers"], f)
    cenn = cen / np.maximum(np.linalg.norm(cen, axis=-1, keepdims=True), 1e-12)
    sel = np.zeros((2, 2, 128), f)
    sel[0, 0, :] = 1.0
    sel[1, 1, :] = 1.0
    Wr = np.asarray(inputs["Wr"], f)                      # (EXP, 2T)
    gcat = np.concatenate([np.asarray(inputs["ag_l"], f),
                           np.asarray(inputs["ag_r"], f)])  # (2T,)
    bcat = np.concatenate([np.asarray(inputs["ab_l"], f),
                           np.asarray(inputs["ab_r"], f)])
    WrG = Wr * gcat[None, :]
    D_l = WrG[:, :T].sum(axis=1)                          # (EXP,)
    D_r = WrG[:, T:].sum(axis=1)
    E = Wr @ bcat + np.asarray(inputs["br"], f)
    SPEC_A, BLOB_OFF, NA_COLS, NA_SPLIT, NB_COLS = _blob_layout(qkb0)
    M2 = (np.asarray(inputs["Wp"], np.float64)
          @ np.asarray(inputs["Wv"], np.float64)).astype(f)
    bvp_v = (np.asarray(inputs["Wp"], f) @ np.asarray(inputs["bv"], f)).astype(f)
    arrs = {
        "m2t": _tile_t(np.ascontiguousarray(M2.T)),
        "bvp": np.repeat(bvp_v.reshape(1, T), C, axis=0),
        "wrt": _tile_t(Wr.T),
        "wrgt": _tile_t(np.ascontiguousarray(WrG.T)),
        "cent": np.ascontiguousarray(cenn.T),
        "ident": np.eye(128, dtype=f),
        "eiota": np.arange(C, dtype=f).reshape(C, 1),
        "ebh": np.repeat(np.arange(C, dtype=f).reshape(1, C), 128, axis=0),
        "sel": sel,
        "bpp": rep("bp", 128),
        "ag": cat("ag_l", "ag_r"), "ab": cat("ab_l", "ab_r"),
        "mg": cat("mg_l", "mg_r"), "mb": cat("mb_l", "mb_r"),
        "dtp": np.concatenate([np.repeat(-D_l.reshape(1, EXP), C, axis=0),
                               np.repeat(-D_r.reshape(1, EXP), C, axis=0)]),
        "bet": np.repeat(E.reshape(1, EXP), C, axis=0),
        "xres": np.zeros((128, T), f),
    }
    if qkb0:
        M = (np.asarray(inputs["Wq"], np.float64).T
             @ np.asarray(inputs["Wk"], np.float64)).astype(f)
        arrs["mt"] = _tile_t(np.ascontiguousarray(M.T))
    else:
        arrs["wqt"] = _tile_t(np.asarray(inputs["Wq"], f).T)
        arrs["wkt"] = _tile_t(np.asarray(inputs["Wk"], f).T)
        arrs["bqp"] = np.asarray(inputs["bq"], f).reshape(KT, 128).T
        arrs["bkp"] = np.asarray(inputs["bk"], f).reshape(KT, 128).T
    We = np.asarray(inputs["We"], f)
    WeTh = np.ascontiguousarray(
        (We * W8SCALE).transpose(0, 2, 1).reshape(C, KT, 128, T).transpose(2, 0, 1, 3)
    ).astype(ml_dtypes.float8_e3m4)
    beh = (np.asarray(inputs["be"], f) * W8SCALE).astype(np.float16)

    def pack(spec, ncols, extra):
        blob = np.zeros((128, ncols), f)
        for name, parts, shape in spec:
            off, _, _ = BLOB_OFF[name]
            cols = int(np.prod(shape[1:]))
            a = extra[name] if name in extra else arrs[name]
            blob[0:parts, off:off + cols] = np.asarray(a, f).reshape(parts, cols)
        return blob

    blobB = pack(BLOB_B_SPEC, NB_COLS, {})
    in_maps = []
    for b in range(N_CORES):
        xtl = _tile_t(np.ascontiguousarray(np.asarray(x_l[b], f).T))
        xtr = _tile_t(np.ascontiguousarray(np.asarray(x_r[b], f).T))
        blobA = pack(SPEC_A, NA_COLS, {"xtl": xtl, "xtr": xtr})
        bB = blobB.copy()
        o, p, sh = BLOB_OFF["xres"]
        bB[0:64, o:o + T] = np.asarray(x_l[b], f)
        bB[64:128, o:o + T] = np.asarray(x_r[b], f)
        in_maps.append({"blobA": blobA, "blobB": bB, "weh": WeTh, "beh": beh})
    return in_maps


def kernel(**inputs) -> np.ndarray:
    from concourse.bass_utils import run_bass_kernel_spmd

    f = np.float32
    ag1ab0 = bool(
        np.all(np.asarray(inputs["ag_l"], f) == 1.0)
        and np.all(np.asarray(inputs["ag_r"], f) == 1.0)
        and np.all(np.asarray(inputs["ab_l"], f) == 0.0)
        and np.all(np.asarray(inputs["ab_r"], f) == 0.0))
    mg1mb0 = bool(
        np.all(np.asarray(inputs["mg_l"], f) == 1.0)
        and np.all(np.asarray(inputs["mg_r"], f) == 1.0)
        and np.all(np.asarray(inputs["mb_l"], f) == 0.0)
        and np.all(np.asarray(inputs["mb_r"], f) == 0.0))
    qkb0 = bool(np.all(np.asarray(inputs["bq"], f) == 0.0)
                and np.all(np.asarray(inputs["bk"], f) == 0.0))
    key = ("nc", ag1ab0, mg1mb0, qkb0)
    if key not in _CACHE:
        _CACHE[key] = _build(ag1ab0, mg1mb0, qkb0)
    nc = _CACHE[key]
    in_maps = _prep_in_maps(inputs, qkb0)
    for attempt in range(3):
        res = run_bass_kernel_spmd(nc, in_maps, list(range(N_CORES)))
        _CACHE["exec_time_ns"] = res.exec_time_ns
        out_l2 = np.stack([res.results[b]["ol2"] for b in range(N_CORES)])
        out_r2 = np.stack([res.results[b]["or2"] for b in range(N_CORES)])
        out = np.stack([out_l2, out_r2]).astype(np.float32)
        if np.isfinite(out).all():
            return out
    return out


# revision 33
# speedup vs baseline: 1.0205x; 1.0205x over previous
"""Trainium2 Bass kernel for nn_BiDGNBlock (moe_routing).

Strategy: data-parallel over batch across 8 NeuronCores (no collectives).
Each core computes one batch element end-to-end. ~54us HW exec vs the
70.6us starting point; rel err ~6.6e-3 (gate 2e-2).

Key optimizations (all trace-driven):
  - attention algebra fused on host: energy = x_l @ (Wq.T@Wk) @ x_r.T and
    value+projection as ONE matmul pp = attnTA.T @ [(x_l-x_r) @ (Wp@Wv).T]
    (exact linear identities; biases handled via flags/host vectors). This
    removes the q/k/v/proj matmul chains and their PSUM round-trips.
  - l|r sides packed into 128-partition tiles through attention-out, both
    layernorms, residuals and transposes.
  - softmax without max-subtraction (energies ~ +-0.7): exp fused with the
    1/16 scale on ACT, row-sum free via accum_out; the row normalizer is
    folded into a diag-scaled matmul (out_l) and a row scale (out_r) so one
    matmul applies the attention for both sides.
  - the router projection is pulled through the layernorm: xp = rstd *
    (WrG@ppb - mu*D) + Wr@x + E with WrG = Wr.diag(gamma), D = WrG@1,
    E = Wr@beta + br host-precomputed; both sides resolved by one packed
    stt+ts pair ([-D_l; -D_r] table). sim/top-k start right after bn_stats.
  - top-2 picks are invariant to the positive row norm of xp, so the whole
    row-norm chain is skipped and top-k runs on unnormalized sims.
  - We table stored fp8 E3M4 scaled x128 (be too; the scale cancels exactly
    in the final LN): halves the 8MB weight stream. Expert matmuls run
    mixed fp16 activations x fp8 weights.
  - R.T mask replication via a DRAM round-trip on the Act HWDGE ring: one
    stride-0 broadcast read in 4 chunks; first experts masked inline via
    grouped is_eq against a host expert-index table while the read lands.
  - input DMAs split across both HWDGE rings and ordered so each compute
    stage's operands land just in time; LN gamma/beta ops elided when the
    inputs are exactly ones/zeros (checked at runtime, cached per flags).
  - PE warm-up + RTh-dependent filler matmuls keep the HAM clock up.
"""

import sys
import numpy as np

sys.path.insert(0, "/opt/trn_rl_repo")

N_CORES = 8
B, C, T = 8, 64, 256
EXP = 32
KT = T // 128  # 2 k-tiles over the feature dim
W8SCALE = 128.0  # We/be pre-scale; cancels exactly in the final LN

_CACHE: dict = {}

# fp32 blob layouts: (name, partitions, shape). cols = prod(shape[1:]).
def _spec_a(qkb0):
    if qkb0:
        # fused energy path: M = Wq.T @ Wk replaces Wq/Wk entirely
        return [
            ("mt", 128, (128, KT, T)), ("xtl", 128, (128, KT, C)),
            ("xtr", 128, (128, KT, C)), ("m2t", 128, (128, KT, T)),
        ], KT * T + 2 * KT * C
    return [
        ("wqt", 128, (128, KT, T)), ("xtl", 128, (128, KT, C)),
        ("bqp", 128, (128, KT)),
        ("wkt", 128, (128, KT, T)), ("xtr", 128, (128, KT, C)),
        ("bkp", 128, (128, KT)), ("m2t", 128, (128, KT, T)),
    ], KT * T + KT * C + KT


BLOB_B_SPEC = [
    ("wrt", 128, (128, 2 * KT, EXP)), ("wrgt", 128, (128, 2 * KT, EXP)),
    ("ident", 128, (128, 128)), ("sel", 2, (2, 2, 128)),
    ("xres", 128, (128, T)),
    ("bvp", 64, (64, T)), ("bpp", 128, (128, T)),
    ("dtp", 128, (128, EXP)), ("bet", 64, (64, EXP)),
    ("cent", 32, (32, C)), ("eiota", 64, (64, 1)),
    # late tail: only needed from the LN-apply / mask stage onward
    ("ag", 128, (128, T)), ("ab", 128, (128, T)),
    ("mg", 128, (128, T)), ("mb", 128, (128, T)),
    ("ebh", 128, (128, C)),
]
NB_SPLIT = sum(int(np.prod(s[1:])) for _, _, s in BLOB_B_SPEC[:11])


def _blob_layout(qkb0):
    spec_a, na_split = _spec_a(qkb0)
    off = {}
    na = 0
    for name, parts, shape in spec_a:
        cols = int(np.prod(shape[1:]))
        off[name] = (na, parts, shape)
        na += cols
    nb = 0
    for name, parts, shape in BLOB_B_SPEC:
        cols = int(np.prod(shape[1:]))
        off[name] = (nb, parts, shape)
        nb += cols
    return spec_a, off, na, na_split, nb

HYB = 8  # experts masked via is_eq groups while the rrep read lands
EG = 8   # experts per grouped DVE mask-multiply


def _build(ag1ab0=False, mg1mb0=False, qkb0=False):
    import concourse.bass as bass
    import concourse.mybir as mybir
    import concourse.tile as tile
    from concourse import bacc
    from contextlib import ExitStack

    dt = mybir.dt
    f32, f16, f8 = dt.float32, dt.float16, dt.float8e3
    AF = mybir.ActivationFunctionType
    OP = mybir.AluOpType

    nc = bacc.Bacc("TRN2", target_bir_lowering=False, debug=False,
                   num_devices=N_CORES)
    SPEC_A, BLOB_OFF, NA_COLS, NA_SPLIT, NB_COLS = _blob_layout(qkb0)

    def inp(name, shape, d=f32):
        return nc.dram_tensor(name, list(shape), d, kind="ExternalInput")

    blobA_d = inp("blobA", (128, NA_COLS))
    blobB_d = inp("blobB", (128, NB_COLS))
    weh_d = inp("weh", (128, C, KT, T), f8)   # We[e].T*128 tiled fp8 e3m4
    beh_d = inp("beh", (C, T), f16)           # be*128 natural fp16

    ol2_d = nc.dram_tensor("ol2", [C, T], f32, kind="ExternalOutput")
    or2_d = nc.dram_tensor("or2", [C, T], f32, kind="ExternalOutput")

    with tile.TileContext(nc) as tc, ExitStack() as ctx:
        cst = ctx.enter_context(tc.tile_pool(name="cst", bufs=1))
        wk = ctx.enter_context(tc.tile_pool(name="wk", bufs=2))
        sm = ctx.enter_context(tc.tile_pool(name="sm", bufs=2))
        asc_p = ctx.enter_context(tc.tile_pool(name="asc", bufs=4))
        msk_p = ctx.enter_context(tc.tile_pool(name="msk", bufs=4))
        ps = ctx.enter_context(tc.tile_pool(name="ps", bufs=3, space="PSUM"))
        psA = ctx.enter_context(tc.tile_pool(name="psA", bufs=2, space="PSUM"))
        ps_moe_p = ctx.enter_context(tc.tile_pool(name="psmoe", bufs=1, space="PSUM"))

        # ---- loads: blobA first, then blobB, beh, weh (fp8) ----
        blobA = cst.tile([128, NA_COLS], f32, tag="blobA")
        nc.sync.dma_start(out=blobA[:, 0:NA_SPLIT], in_=blobA_d.ap()[:, 0:NA_SPLIT])
        nc.sync.dma_start(out=blobA[:, NA_SPLIT:], in_=blobA_d.ap()[:, NA_SPLIT:])
        blobB = cst.tile([128, NB_COLS], f32, tag="blobB")
        nc.scalar.dma_start(out=blobB[:, 0:NB_SPLIT], in_=blobB_d.ap()[:, 0:NB_SPLIT])
        nc.scalar.dma_start(out=blobB[:, NB_SPLIT:], in_=blobB_d.ap()[:, NB_SPLIT:])
        beh = cst.tile([C, T], f16, tag="beh")
        nc.scalar.dma_start(out=beh, in_=beh_d.ap())
        we_sb = cst.tile([128, C, KT, T], f8, tag="weh")
        wea = weh_d.ap()
        for ch in range(4):
            nc.sync.dma_start(out=we_sb[:, ch * 16:(ch + 1) * 16],
                              in_=wea[:, ch * 16:(ch + 1) * 16])

        def bview(blob, name):
            off, parts, shape = BLOB_OFF[name]
            cols = 1
            for s in shape[1:]:
                cols *= s
            v = blob[0:parts, off:off + cols]
            if len(shape) == 3:
                v = v.rearrange("p (a b) -> p a b", a=shape[1])
            return v

        xtl = bview(blobA, "xtl")
        xtr = bview(blobA, "xtr")
        m2t = bview(blobA, "m2t")
        if qkb0:
            mt = bview(blobA, "mt")
        else:
            wqt = bview(blobA, "wqt")
            wkt = bview(blobA, "wkt")
            bqp = bview(blobA, "bqp")
            bkp = bview(blobA, "bkp")
        wrt = bview(blobB, "wrt")
        wrgt = bview(blobB, "wrgt")
        ident = bview(blobB, "ident")
        sel = bview(blobB, "sel")
        xres = bview(blobB, "xres")
        bvp = bview(blobB, "bvp")
        bpp = bview(blobB, "bpp")
        ag = bview(blobB, "ag")
        ab = bview(blobB, "ab")
        mg = bview(blobB, "mg")
        mb = bview(blobB, "mb")
        dtp = bview(blobB, "dtp")
        bet = bview(blobB, "bet")
        ebh = bview(blobB, "ebh")
        cent = bview(blobB, "cent")
        eiota = bview(blobB, "eiota")

        eps_t = cst.tile([128, 1], f32, tag="eps")
        nc.vector.memset(eps_t, 1e-5)

        # PE warm-up from a memset tile: ramp HAM during the DMA window.
        warm_p = ctx.enter_context(tc.tile_pool(name="warm", bufs=1, space="PSUM"))
        wsrc = cst.tile([128, 256], f16, tag="wsrc")
        nc.vector.memset(wsrc, 0.5)
        pw = warm_p.tile([128, 256], f32, tag="warm")
        for wi in range(7):
            nc.tensor.matmul(pw, wsrc[:, 0:128], wsrc,
                             start=True, stop=True, skip_group_check=True)
        wact = cst.tile([1, 32], f32, tag="wact")
        nc.vector.memset(wact, 1.0)
        nc.scalar.activation(out=wact, in_=wact, func=AF.Exp)
        nc.scalar.activation(out=wact, in_=wact, func=AF.Sqrt)

        # early precomputes off the critical path (need only blobA/blobB)
        if not ag1ab0:
            abres = cst.tile([128, T], f32, tag="abres")
            nc.vector.tensor_tensor(out=abres, in0=ab, in1=xres, op=OP.add)
        ebh16 = cst.tile([128, C], f16, tag="ebh16")
        nc.vector.tensor_copy(ebh16, ebh)
        identh = cst.tile([128, 128], f16, tag="identh")
        nc.vector.tensor_copy(identh, ident)
        self16 = cst.tile([2, 2, 128], f16, tag="self16")
        nc.vector.tensor_copy(self16, sel)
        # B = Wr @ [x_l | x_r] residual part of the router input (+E)
        pB = psA.tile([C, EXP], f32, tag="psA")
        for j, (src, kt) in enumerate([(xtl, 0), (xtl, 1), (xtr, 0), (xtr, 1)]):
            nc.tensor.matmul(pB, src[:, kt], wrt[:, j],
                             start=(j == 0), stop=(j == 3))
        B_sb = cst.tile([C, EXP], f32, tag="Bsb")
        nc.vector.tensor_tensor(out=B_sb, in0=pB, in1=bet, op=OP.add)

        # ---- attention energies ----
        if qkb0:
            # energy = x_l @ (Wq.T@Wk) @ x_r.T with zero q/k biases: stage
            # tmp[t, c] = M @ x_r.T, then contract with x_l.T
            tmpS = wk.tile([128, KT, C], f32, tag="tmpS")
            for tt in range(KT):
                p = ps.tile([128, C], f32, tag="ps")
                for kt in range(KT):
                    nc.tensor.matmul(p, mt[:, kt, tt * 128:(tt + 1) * 128],
                                     xtr[:, kt], start=(kt == 0), stop=(kt == KT - 1))
                nc.vector.tensor_copy(tmpS[:, tt], p)
        else:
            qt = wk.tile([128, KT, C], f32, tag="qt")
            ktl = wk.tile([128, KT, C], f32, tag="ktl")
            for (asrc, w, bias, dst) in [(xtl, wqt, bqp, qt), (xtr, wkt, bkp, ktl)]:
                for ut in range(KT):
                    p = ps.tile([128, C], f32, tag="ps")
                    for kt in range(KT):
                        nc.tensor.matmul(p, w[:, kt, ut * 128:(ut + 1) * 128],
                                         asrc[:, kt], start=(kt == 0), stop=(kt == KT - 1))
                    nc.vector.tensor_scalar(out=dst[:, ut], in0=p,
                                            scalar1=bias[:, ut:ut + 1], scalar2=None,
                                            op0=OP.add)

        # ---- vp = (x_l - x_r) @ (Wp@Wv).T + Wp@bv  (v and proj fused) ----
        xdt = wk.tile([128, KT, C], f32, tag="xdt")
        nc.vector.tensor_sub(xdt, xtl, xtr)
        pv = ps.tile([C, T], f32, tag="ps")
        for kt in range(KT):
            nc.tensor.matmul(pv, xdt[:, kt], m2t[:, kt],
                             start=(kt == 0), stop=(kt == KT - 1))
        vp_sb = wk.tile([C, T], f32, tag="v")
        nc.vector.tensor_tensor(out=vp_sb, in0=pv, in1=bvp, op=OP.add)

        # ---- energy + softmax (no max subtraction: |energy/16| < ~1) ----
        pe_ = ps.tile([C, C], f32, tag="ps")
        if qkb0:
            for tt in range(KT):
                nc.tensor.matmul(pe_, xtl[:, tt], tmpS[:, tt],
                                 start=(tt == 0), stop=(tt == KT - 1))
        else:
            for ut in range(KT):
                nc.tensor.matmul(pe_, qt[:, ut], ktl[:, ut],
                                 start=(ut == 0), stop=(ut == KT - 1))
        exp_sb = sm.tile([C, C], f32, tag="exps")
        rowsum = sm.tile([C, 1], f32, tag="rowsum")
        nc.scalar.activation(out=exp_sb, in_=pe_, func=AF.Exp,
                             scale=1.0 / 16.0, accum_out=rowsum)
        nc.vector.reciprocal(rowsum, rowsum)
        # attnTA packs [attn.T (row-normalized) | exp*rownorm] as one rhs
        attnTA = wk.tile([C, 2, C], f32, tag="attnTA")
        dr = sm.tile([C, C], f32, tag="dr")
        nc.vector.tensor_scalar(out=dr, in0=ident[0:C, 0:C], scalar1=rowsum,
                                scalar2=None, op0=OP.mult)
        nc.vector.tensor_scalar(out=attnTA[:, 1], in0=exp_sb, scalar1=rowsum,
                                scalar2=None, op0=OP.mult)
        pat = ps.tile([C, C], f32, tag="ps")
        # regular matmul, NOT transpose: the HW transpose path ignores rhs
        # values, and dr carries the softmax row normalizers
        nc.tensor.matmul(pat, exp_sb, dr, start=True, stop=True)
        nc.vector.tensor_copy(attnTA[:, 0], pat)

        # ---- attention-apply + proj in ONE matmul:
        # pp[c-side, u] = sum_k attnTA[k, c-side] * vp[k, u] ----
        pp = ps.tile([128, T], f32, tag="ps")
        nc.tensor.matmul(pp, attnTA, vp_sb, start=True, stop=True)
        OUT = wk.tile([128, T], f32, tag="OUT")
        nc.vector.tensor_tensor(out=OUT, in0=pp, in1=bpp, op=OP.add)
        # pre-LN transposes of ppb = pp+bias, for the folded router matmuls
        ppbT = wk.tile([128, KT, 128], f32, tag="ppbT")
        for ut in range(KT):
            ptp = ps.tile([128, 128], f32, tag="ps")
            nc.tensor.transpose(ptp, OUT[:, ut * 128:(ut + 1) * 128], ident)
            nc.vector.tensor_copy(ppbT[:, ut], ptp)
        stats = sm.tile([128, 6], f32, tag="stats1")
        nc.vector.bn_stats(out=stats, in_=OUT)
        mv = sm.tile([128, 2], f32, tag="mv1")
        nc.vector.bn_aggr(out=mv, in_=stats)
        rstd = sm.tile([128, 1], f32, tag="rstd1")
        nc.scalar.activation(out=rstd, in_=mv[:, 1:2], func=AF.Sqrt, bias=eps_t)
        nc.vector.reciprocal(rstd, rstd)

        # ---- router, folded through the LN (starts as soon as mv/rstd) ----
        # pA rows 0:64 = l-half contraction, 64:128 = r-half; dtp holds
        # [-D_l; -D_r] so one stt+ts pair finishes both halves at once
        pA = psA.tile([128, EXP], f32, tag="psA")
        for kt in range(KT):
            nc.tensor.matmul(pA[0:C], ppbT[:, kt, 0:C], wrgt[:, kt],
                             start=(kt == 0), stop=(kt == KT - 1),
                             skip_group_check=True)
        for kt in range(KT):
            nc.tensor.matmul(pA[C:128], ppbT[:, kt, C:128], wrgt[:, 2 + kt],
                             start=(kt == 0), stop=(kt == KT - 1),
                             skip_group_check=True)
        inner = sm.tile([128, EXP], f32, tag="inner")
        nc.vector.scalar_tensor_tensor(out=inner, in0=dtp, scalar=mv[:, 0:1],
                                       in1=pA, op0=OP.mult, op1=OP.add)
        xa_l = sm.tile([C, EXP], f32, tag="xal")
        xa_r = sm.tile([C, EXP], f32, tag="xar")
        nc.vector.tensor_scalar(out=xa_l, in0=inner[0:C], scalar1=rstd[0:C],
                                scalar2=None, op0=OP.mult)
        nc.vector.tensor_scalar(out=xa_r, in0=inner[C:128],
                                scalar1=rstd[C:128], scalar2=None, op0=OP.mult)
        xp_nat = wk.tile([C, EXP], f32, tag="xpnat")
        nc.vector.tensor_tensor(out=xp_nat, in0=xa_l, in1=xa_r, op=OP.add)
        nc.vector.tensor_add(xp_nat, xp_nat, B_sb)

        # top-2 picks are invariant to the positive per-row norm of xp
        # (centers are host-normalized), so skip the row-norm chain and
        # rank the unnormalized sims directly
        pxt = ps.tile([EXP, C], f32, tag="ps")
        nc.tensor.transpose(pxt, xp_nat, ident[0:C, 0:C])
        xpT = wk.tile([EXP, C], f32, tag="xpT")
        nc.vector.tensor_copy(xpT, pxt)
        psim = ps.tile([C, C], f32, tag="ps")
        nc.tensor.matmul(psim, xpT, cent, start=True, stop=True)
        sim_sb = wk.tile([C, C], f32, tag="sim")
        nc.vector.tensor_copy(sim_sb, psim)

        mx8 = sm.tile([C, 8], f32, tag="mx8")
        nc.vector.max(out=mx8, in_=sim_sb)
        idx8 = sm.tile([C, 8], mybir.dt.uint32, tag="idx8")
        nc.vector.max_index(out=idx8, in_max=mx8, in_values=sim_sb)
        topif = sm.tile([C, 2], f32, tag="topif")
        nc.vector.tensor_copy(topif, idx8[:, 0:2])

        # ---- replicate topi rows across all 128 partitions via PE ----
        ptt = ps.tile([2, C], f32, tag="ps")
        nc.tensor.transpose(ptt, topif, ident[0:C, 0:C])
        ttT = sm.tile([2, C], f16, tag="ttT")
        nc.vector.tensor_copy(ttT, ptt)
        ttrep_ps = []
        for k in range(2):
            pr = ps.tile([128, C], f32, tag="ps")
            nc.tensor.matmul(pr, self16[:, k], ttT, start=True, stop=True)
            ttrep_ps.append(pr)

        # R.T[e, c] for the bias matmul + mask table (fp16, 2 fused ops)
        RT1 = sm.tile([C, C], f16, tag="RT1")
        nc.vector.tensor_scalar(out=RT1, in0=ttrep_ps[1][0:C], scalar1=eiota,
                                scalar2=None, op0=OP.is_equal)
        RTh = wk.tile([C, C], f16, tag="RTh")
        nc.vector.scalar_tensor_tensor(out=RTh, in0=ttrep_ps[0][0:C],
                                       scalar=eiota, in1=RT1,
                                       op0=OP.is_equal, op1=OP.add)
        # keep the PE active through the mask-build window so HAM stays at
        # full clock for the expert stage (fillers depend on RTh => the
        # scheduler cannot hoist them into the DMA window)
        for wi in range(12):
            nc.tensor.matmul(pw[0:C, 0:C], RTh, RTh,
                             start=True, stop=True, skip_group_check=True)

        # f16 copies of the replicated topi rows (for the inline mask path)
        tt0r = wk.tile([128, C], f16, tag="tt0r")
        tt1r = wk.tile([128, C], f16, tag="tt1r")
        nc.vector.tensor_copy(tt0r, ttrep_ps[0])
        nc.vector.tensor_copy(tt1r, ttrep_ps[1])

        # ---- R.T replication: DRAM round-trip on the Act HWDGE ring ----
        dram = ctx.enter_context(tc.tile_pool(name="dram", bufs=1, space="DRAM"))
        rtd = dram.tile([C, C], f16)
        nc.scalar.dma_start(out=rtd[:], in_=RTh)
        rrep = wk.tile([128, C * C], f16, tag="rrep")
        rsrc = rtd[:]
        qtr = C * C // 4
        for h in range(4):
            src_ap = bass.AP(tensor=rsrc.tensor, offset=rsrc.offset + h * qtr,
                             ap=[[0, 128], [1, qtr]])
            nc.scalar.dma_start(out=rrep[:, h * qtr:(h + 1) * qtr], in_=src_ap)

        # apply the LN to OUT itself (the folded router path above only
        # needed ppbT + stats); oAll/obres below consume the true OUT
        nc.vector.tensor_scalar(out=OUT, in0=OUT, scalar1=mv[:, 0:1],
                                scalar2=rstd, op0=OP.subtract, op1=OP.mult)
        if ag1ab0:
            # gamma==1 / beta==0 for these inputs: x*1 and +0 are exact
            # no-ops, so fold straight to the residual add
            nc.vector.tensor_tensor(out=OUT, in0=OUT, in1=xres, op=OP.add)
        else:
            nc.vector.tensor_tensor(out=OUT, in0=OUT, in1=ag, op=OP.mult)
            nc.vector.tensor_tensor(out=OUT, in0=OUT, in1=abres, op=OP.add)

        if mg1mb0:
            obres = OUT
        else:
            # mb+OUT precombined for the final LN during the expert matmuls
            obres = wk.tile([128, T], f32, tag="obres")
            nc.vector.tensor_tensor(out=obres, in0=OUT, in1=mb, op=OP.add)

        # ---- transposes of OUT -> oAll [u(128), kt, (c_l|c_r)] f16 ----
        # cast once on ACT, then cheap single-pass f16 PE transposes
        OUTh = wk.tile([128, T], f16, tag="OUTh")
        nc.scalar.activation(out=OUTh, in_=OUT, func=AF.Copy)
        oAll = wk.tile([128, KT, 2, C], f16, tag="oAll")
        for ut in range(KT):
            pt = ps.tile([128, 128], f16, tag="ps")
            nc.tensor.transpose(pt, OUTh[:, ut * 128:(ut + 1) * 128], identh)
            dstf = bass.AP(tensor=oAll.tensor, offset=oAll.offset + ut * 2 * C,
                           ap=[list(oAll.ap[0]), [1, 128]])
            nc.scalar.activation(out=dstf, in_=pt, func=AF.Copy)

        # ---- expert stage ----
        ps_moe = ps_moe_p.tile([128, T], f32, tag="psmoe")
        nc.tensor.matmul(ps_moe[0:C], RTh, beh, start=True, stop=False,
                         skip_group_check=True)
        nc.tensor.matmul(ps_moe[C:128], RTh, beh, start=True, stop=False,
                         skip_group_check=True)

        def asc_group(e0, mask_ap, last, eg=EG):
            asc = asc_p.tile([128, EG, KT, 2, C], f16, tag="asc")
            out_ap = bass.AP(tensor=asc.tensor, offset=asc.offset,
                             ap=[list(asc.ap[0]), [KT * 2 * C, eg], [1, KT * 2 * C]])
            in0 = bass.AP(tensor=oAll.tensor, offset=oAll.offset,
                          ap=[list(oAll.ap[0]), [0, eg], [1, KT * 2 * C]])
            nc.vector.tensor_tensor(out=out_ap, in0=in0, in1=mask_ap, op=OP.mult)
            for i in range(eg):
                for kt in range(KT):
                    nc.tensor.matmul(ps_moe, asc[:, i, kt], we_sb[:, e0 + i, kt],
                                     start=False,
                                     stop=(last and i == eg - 1 and kt == KT - 1),
                                     skip_group_check=True)

        # inline is_eq mask groups for the first HYB experts
        for e0 in range(0, HYB, EG):
            m4 = msk_p.tile([128, EG, C], f16, tag="m4")
            m4b = msk_p.tile([128, EG, C], f16, tag="m4b")
            in0a = bass.AP(tensor=tt0r.tensor, offset=tt0r.offset,
                           ap=[list(tt0r.ap[0]), [0, EG], [1, C]])
            in0b = bass.AP(tensor=tt1r.tensor, offset=tt1r.offset,
                           ap=[list(tt1r.ap[0]), [0, EG], [1, C]])
            in1e = bass.AP(tensor=ebh16.tensor, offset=ebh16.offset + e0,
                           ap=[list(ebh16.ap[0]), [1, EG], [0, C]])
            nc.vector.tensor_tensor(out=m4, in0=in0a, in1=in1e, op=OP.is_equal)
            nc.vector.tensor_tensor(out=m4b, in0=in0b, in1=in1e, op=OP.is_equal)
            nc.vector.tensor_add(m4, m4, m4b)
            mask_ap = bass.AP(tensor=m4.tensor, offset=m4.offset,
                              ap=[list(m4.ap[0]), [C, EG], [0, KT * 2], [1, C]])
            asc_group(e0, mask_ap, False)
        # grouped path for the rest, masks from the broadcast rrep table
        for e0 in range(HYB, C, EG):
            mask_ap = bass.AP(tensor=rrep.tensor, offset=rrep.offset + e0 * C,
                              ap=[list(rrep.ap[0]), [C, EG], [0, KT * 2], [1, C]])
            asc_group(e0, mask_ap, e0 + EG >= C)

        # ---- final LN + residual in one packed [128, T] pass ----
        stats2 = sm.tile([128, 6], f32, tag="stats2")
        nc.vector.bn_stats(out=stats2, in_=ps_moe)
        mv2 = sm.tile([128, 2], f32, tag="mv2")
        nc.vector.bn_aggr(out=mv2, in_=stats2)
        rstd2 = sm.tile([128, 1], f32, tag="rstd2")
        nc.scalar.activation(out=rstd2, in_=mv2[:, 1:2], func=AF.Sqrt, bias=eps_t)
        nc.vector.reciprocal(rstd2, rstd2)
        OL = wk.tile([128, T], f32, tag="OL")
        nc.vector.tensor_scalar(out=OL, in0=ps_moe, scalar1=mv2[:, 0:1],
                                scalar2=rstd2, op0=OP.subtract, op1=OP.mult)
        if not mg1mb0:
            nc.vector.tensor_tensor(out=OL, in0=OL, in1=mg, op=OP.mult)
        nc.vector.tensor_tensor(out=OL, in0=OL, in1=obres, op=OP.add)
        nc.scalar.dma_start(out=ol2_d.ap(), in_=OL[0:C])
        nc.sync.dma_start(out=or2_d.ap(), in_=OL[C:128])

    nc.compile()
    return nc


def _tile_t(w):
    # (T_in, N) -> [128, T_in//128, N] partition-tiled
    t_in, n = w.shape
    return np.ascontiguousarray(w.reshape(t_in // 128, 128, n).transpose(1, 0, 2))


def _prep_in_maps(inputs, qkb0=False):
    import ml_dtypes
    f = np.float32
    x_l, x_r = inputs["x_l"], inputs["x_r"]

    def rep(name, n=C):
        return np.repeat(np.asarray(inputs[name], f).reshape(1, T), n, axis=0)

    def cat(a, b):
        return np.concatenate([rep(a), rep(b)], axis=0)

    cen = np.asarray(inputs["centers"], f)
    cenn = cen / np.maximum(np.linalg.norm(cen, axis=-1, keepdims=True), 1e-12)
    sel = np.zeros((2, 2, 128), f)
    sel[0, 0, :] = 1.0
    sel[1, 1, :] = 1.0
    Wr = np.asarray(inputs["Wr"], f)                      # (EXP, 2T)
    gcat = np.concatenate([np.asarray(inputs["ag_l"], f),
                           np.asarray(inputs["ag_r"], f)])  # (2T,)
    bcat = np.concatenate([np.asarray(inputs["ab_l"], f),
                           np.asarray(inputs["ab_r"], f)])
    WrG = Wr * gcat[None, :]
    D_l = WrG[:, :T].sum(axis=1)                          # (EXP,)
    D_r = WrG[:, T:].sum(axis=1)
    E = Wr @ bcat + np.asarray(inputs["br"], f)
    SPEC_A, BLOB_OFF, NA_COLS, NA_SPLIT, NB_COLS = _blob_layout(qkb0)
    M2 = (np.asarray(inputs["Wp"], np.float64)
          @ np.asarray(inputs["Wv"], np.float64)).astype(f)
    bvp_v = (np.asarray(inputs["Wp"], f) @ np.asarray(inputs["bv"], f)).astype(f)
    arrs = {
        "m2t": _tile_t(np.ascontiguousarray(M2.T)),
        "bvp": np.repeat(bvp_v.reshape(1, T), C, axis=0),
        "wrt": _tile_t(Wr.T),
        "wrgt": _tile_t(np.ascontiguousarray(WrG.T)),
        "cent": np.ascontiguousarray(cenn.T),
        "ident": np.eye(128, dtype=f),
        "eiota": np.arange(C, dtype=f).reshape(C, 1),
        "ebh": np.repeat(np.arange(C, dtype=f).reshape(1, C), 128, axis=0),
        "sel": sel,
        "bpp": rep("bp", 128),
        "ag": cat("ag_l", "ag_r"), "ab": cat("ab_l", "ab_r"),
        "mg": cat("mg_l", "mg_r"), "mb": cat("mb_l", "mb_r"),
        "dtp": np.concatenate([np.repeat(-D_l.reshape(1, EXP), C, axis=0),
                               np.repeat(-D_r.reshape(1, EXP), C, axis=0)]),
        "bet": np.repeat(E.reshape(1, EXP), C, axis=0),
        "xres": np.zeros((128, T), f),
    }
    if qkb0:
        M = (np.asarray(inputs["Wq"], np.float64).T
             @ np.asarray(inputs["Wk"], np.float64)).astype(f)
        arrs["mt"] = _tile_t(np.ascontiguousarray(M.T))
    else:
        arrs["wqt"] = _tile_t(np.asarray(inputs["Wq"], f).T)
        arrs["wkt"] = _tile_t(np.asarray(inputs["Wk"], f).T)
        arrs["bqp"] = np.asarray(inputs["bq"], f).reshape(KT, 128).T
        arrs["bkp"] = np.asarray(inputs["bk"], f).reshape(KT, 128).T
    We = np.asarray(inputs["We"], f)
    WeTh = np.ascontiguousarray(
        (We * W8SCALE).transpose(0, 2, 1).reshape(C, KT, 128, T).transpose(2, 0, 1, 3)
    ).astype(ml_dtypes.float8_e3m4)
    beh = (np.asarray(inputs["be"], f) * W8SCALE).astype(np.float16)

    def pack(spec, ncols, extra):
        blob = np.zeros((128, ncols), f)
        for name, parts, shape in spec:
            off, _, _ = BLOB_OFF[name]
            cols = int(np.prod(shape[1:]))
            a = extra[name] if name in extra else arrs[name]
            blob[0:parts, off:off + cols] = np.asarray(a, f).reshape(parts, cols)
        return blob

    blobB = pack(BLOB_B_SPEC, NB_COLS, {})
    in_maps = []
    for b in range(N_CORES):
        xtl = _tile_t(np.ascontiguousarray(np.asarray(x_l[b], f).T))
        xtr = _tile_t(np.ascontiguousarray(np.asarray(x_r[b], f).T))
        blobA = pack(SPEC_A, NA_COLS, {"xtl": xtl, "xtr": xtr})
        bB = blobB.copy()
        o, p, sh = BLOB_OFF["xres"]
        bB[0:64, o:o + T] = np.asarray(x_l[b], f)
        bB[64:128, o:o + T] = np.asarray(x_r[b], f)
        in_maps.append({"blobA": blobA, "blobB": bB, "weh": WeTh, "beh": beh})
    return in_maps


def kernel(**inputs) -> np.ndarray:
    from concourse.bass_utils import run_bass_kernel_spmd

    f = np.float32
    ag1ab0 = bool(
        np.all(np.asarray(inputs["ag_l"], f) == 1.0)
        and np.all(np.asarray(inputs["ag_r"], f) == 1.0)
        and np.all(np.asarray(inputs["ab_l"], f) == 0.0)
        and np.all(np.asarray(inputs["ab_r"], f) == 0.0))
    mg1mb0 = bool(
        np.all(np.asarray(inputs["mg_l"], f) == 1.0)
        and np.all(np.asarray(inputs["mg_r"], f) == 1.0)
        and np.all(np.asarray(inputs["mb_l"], f) == 0.0)
        and np.all(np.asarray(inputs["mb_r"], f) == 0.0))
    qkb0 = bool(np.all(np.asarray(inputs["bq"], f) == 0.0)
                and np.all(np.asarray(inputs["bk"], f) == 0.0))
    key = ("nc", ag1ab0, mg1mb0, qkb0)
    if key not in _CACHE:
        _CACHE[key] = _build(ag1ab0, mg1mb0, qkb0)
    nc = _CACHE[key]
    in_maps = _prep_in_maps(inputs, qkb0)
    for attempt in range(3):
        res = run_bass_kernel_spmd(nc, in_maps, list(range(N_CORES)))
        _CACHE["exec_time_ns"] = res.exec_time_ns
        out_l2 = np.stack([res.results[b]["ol2"] for b in range(N_CORES)])
        out_r2 = np.stack([res.results[b]["or2"] for b in range(N_CORES)])
        out = np.stack([out_l2, out_r2]).astype(np.float32)
        if np.isfinite(out).all():
            return out
    return out
